# revision 7
# baseline (speedup 1.0000x reference)
"""Trainium2 Bass kernel for nn_ConvolutionalAttention_3015067042131.

Math (reference.py):
  x [16,128,64,64] f32; x1 = x[:, :64], x2 = x[:, 64:]
  pooled = mean(x1, HW); h = gelu(pooled @ w1.T + b1); dyn = (h @ w2.T + b2) -> [B,64,9]
  x1_dyn = per-(batch,channel) 3x3 depthwise conv of x1 with dyn
  x1_lk  = conv2d(x1, lk_filter[64,64,13,13], SAME)
  out = concat([x1_lk + x1_dyn, x2], ch)

Strategy:
  * The tiny MLP (dyn) is computed on host in float64 (0.0007% of FLOPs).
  * The dynamic depthwise 3x3 is folded into the 13x13 conv weights as
    per-batch diagonal additions on the central 3x3 taps (3x3 tap (u,v)
    == 13x13 tap (u+5, v+5)); the device runs ONE dense 13x13 conv.
  * Conv as shift-and-matmul with K-dim tap pairing: SBUF partitions
    0-63 hold the zero-padded image, 64-127 hold it shifted one column
    left, so taps (i,2p) and (i,2p+1) fuse into one K=128 matmul
    (78 tiles).  The 13 leftover j=12 taps are paired ROW-wise via a
    second small buffer xpr whose partitions 64-127 hold the padded
    image shifted UP one row: taps (2k,12)+(2k+1,12) fuse (6 tiles),
    (12,12) rides alone (1 tile).  85 tiles total vs the optimal
    ceil(169/2)=85 -> 99.4% K-packing efficiency.
  * Output pixels processed in 8 chunks of 512 (8 rows). Chunk pairs run
    CONCURRENTLY in the two PE column halves via tile_position (0,0) /
    (0,64) writing PSUM partitions 0-63 / 64-127 (measured 2x).
  * fp16 operands (measured end-to-end rel err ~3e-4; f32r is broken in
    this stack and fp32 runs at 1/4 rate). Output stored as f16
    (adds ~<5e-4 elementwise rounding, halves output DMA bytes).
  * Startup critical path (was 24.7us to first real matmul): image DMAs
    are issued BEFORE the weights and split row-wise; weight DMAs are
    issued on the scalar engine's hardware DGE queue so they don't
    serialize behind the image issues on sync; the shared-weight DMA is
    split in 3 so early tap tiles land first; padded-layout interior
    copies run on vector (the scalar-engine copy was 3x slower),
    border memsets on gpsimd/vector.
  * Sharding: data-parallel over batch, 2 batches per core on 8 cores.
    x2 passthrough is host-side (no device work).
"""
import math

import numpy as np

B, C, H, W = 16, 128, 64, 64
PDIM, SK, LK = 64, 3, 13
PAD = LK // 2  # 6
HP, WP = H + 2 * PAD, W + 2 * PAD  # 76, 76
NCORES = 8
BPC = B // NCORES  # batches per core
NP = 85            # 78 col-pair tiles + 6 row-pair j=12 tiles + 1 single
NCHUNK = 8         # 512-pixel chunks per image
CHUNK = H * W // NCHUNK  # 512

# pair tile t = i*6 + p (t<78): taps (i,2p) half-A / (i,2p+1) half-B.
# c12 tile t = 78+k (k<6): taps (2k,12) half-A / (2k+1,12) half-B (xpr).
# t = 84: tap (12,12) half-A only (half-B weight zero).

# central 3x3 taps (i,j in 5..7): j=5 -> half-B of tile i*6+2,
# j=6/7 -> halves A/B of tile i*6+3; those 6 tiles are per-batch.
_MOD_TILES = [5 * 6 + 2, 6 * 6 + 2, 7 * 6 + 2, 5 * 6 + 3, 6 * 6 + 3, 7 * 6 + 3]
_MOD_SLOT = {t: s for s, t in enumerate(_MOD_TILES)}

# wsh DMA split points (tiles): early tap tiles must land first
_WCHUNKS = [(0, 14), (14, 47), (47, NP)]

_ERF = np.vectorize(math.erf, otypes=[np.float64])

_CACHED_NC = None


def _build_nc():
    import concourse.mybir as mybir
    import concourse.tile as tile
    from concourse import bacc

    f32 = mybir.dt.float32
    f16 = mybir.dt.float16

    nc = bacc.Bacc(None, target_bir_lowering=False)
    xs = nc.dram_tensor("xs", [BPC, PDIM, H, W], f16, kind="ExternalInput")
    wsh = nc.dram_tensor("wsh", [128, NP * 64], f16, kind="ExternalInput")
    wmod = nc.dram_tensor("wmod", [BPC, 128, 6 * 64], f16, kind="ExternalInput")
    # chunk-major output: row ci*64+ch holds output rows 8ci..8ci+7 of ch
    y = nc.dram_tensor("y", [BPC, NCHUNK * 64, CHUNK], f16, kind="ExternalOutput")

    with tile.TileContext(nc) as tc:
        with (
            tc.tile_pool(name="wpool", bufs=1) as wpool,
            tc.tile_pool(name="wmpool", bufs=2) as wmpool,
            tc.tile_pool(name="xstpool", bufs=2) as xstpool,
            tc.tile_pool(name="xppool", bufs=2) as xppool,
            tc.tile_pool(name="xprpool", bufs=2) as xprpool,
            tc.tile_pool(name="opool", bufs=3) as opool,
            tc.tile_pool(name="pspool", bufs=4, space="PSUM") as pspool,
        ):
            # ---- warmup scratch (gpsimd memset so vector stays free) ----
            scratch = wpool.tile([128, CHUNK], f16)
            nc.gpsimd.memset(scratch[:], 0.0)

            wsh_sb = wpool.tile([128, NP * 64], f16)
            wm = [wmpool.tile([128, 6 * 64], f16, name=f"wm{b}") for b in range(BPC)]
            xst = [xstpool.tile([128, H, W], f16, name=f"xst{b}") for b in range(BPC)]
            xp = [xppool.tile([128, HP, WP], f16, name=f"xp{b}") for b in range(BPC)]
            xpr = [
                xprpool.tile([128, HP, 64], f16, name=f"xpr{b}") for b in range(BPC)
            ]

            # ---- DMA issue: ONE hardware-DGE queue (sync) in strict
            # priority order — a second issuing queue (scalar) makes the
            # DMA engines interleave weight bytes with the b0 image and
            # delays the copy the first matmul waits on (measured +2.5us).
            nc.sync.dma_start(out=xst[0][0:64, 0:32, :], in_=xs[0, :, 0:32, :])
            nc.sync.dma_start(out=xst[0][64:128, 0:32, :], in_=xs[0, :, 0:32, :])
            nc.sync.dma_start(
                out=wsh_sb[:, _WCHUNKS[0][0] * 64 : _WCHUNKS[0][1] * 64],
                in_=wsh[:, _WCHUNKS[0][0] * 64 : _WCHUNKS[0][1] * 64],
            )
            nc.sync.dma_start(out=xst[0][0:64, 32:64, :], in_=xs[0, :, 32:64, :])
            nc.sync.dma_start(out=xst[0][64:128, 32:64, :], in_=xs[0, :, 32:64, :])
            nc.sync.dma_start(out=wm[0][:], in_=wmod[0, :, :])
            for c0, c1 in _WCHUNKS[1:]:
                nc.sync.dma_start(
                    out=wsh_sb[:, c0 * 64 : c1 * 64], in_=wsh[:, c0 * 64 : c1 * 64]
                )
            nc.sync.dma_start(out=xst[1][0:64, :, :], in_=xs[1, :, :, :])
            nc.sync.dma_start(out=xst[1][64:128, :, :], in_=xs[1, :, :, :])
            nc.sync.dma_start(out=wm[1][:], in_=wmod[1, :, :])

            # ---- PE warmup: junk matmuls so the HAM clock ramps while the
            # input DMAs are in flight.
            ps_warm = pspool.tile([128, CHUNK], mybir.dt.float32, name="ps_warm", bufs=1)
            for wi in range(5):
                nc.tensor.matmul(
                    ps_warm[0:64, :],
                    lhsT=scratch[:, 0:64],
                    rhs=scratch[:, :],
                    start=(wi == 0),
                    stop=(wi == 4),
                    skip_group_check=True,
                )

            # ---- xp borders; for b0 split vector/gpsimd so the vector is
            # free the moment the image DMA lands.
            def xp_borders(eng, t, eng2=None):
                e2 = eng2 or eng
                eng.memset(t[:, 0:PAD, :], 0.0)
                e2.memset(t[:, PAD + H :, :], 0.0)
                eng.memset(t[0:64, PAD : PAD + H, 0:PAD], 0.0)
                e2.memset(t[0:64, PAD : PAD + H, PAD + W :], 0.0)
                eng.memset(t[64:128, PAD : PAD + H, 0 : PAD - 1], 0.0)
                e2.memset(t[64:128, PAD : PAD + H, PAD - 1 + W :], 0.0)

            def xpr_borders(eng, t):
                # half-A: padded rows at col offset 12; half-B same shifted
                # up one row.  cols 58:64 map past the padded width -> zero.
                eng.memset(t[0:64, 0:PAD, :], 0.0)
                eng.memset(t[0:64, PAD + H :, :], 0.0)
                eng.memset(t[0:64, PAD : PAD + H, 58:64], 0.0)
                eng.memset(t[64:128, 0 : PAD - 1, :], 0.0)
                eng.memset(t[64:128, PAD - 1 + H :, :], 0.0)
                eng.memset(t[64:128, PAD - 1 : PAD - 1 + H, 58:64], 0.0)

            xp_borders(nc.vector, xp[0], eng2=nc.gpsimd)
            # remaining borders are needed much later -> gpsimd
            xpr_borders(nc.gpsimd, xpr[0])
            xp_borders(nc.gpsimd, xp[1])
            xpr_borders(nc.gpsimd, xpr[1])

            # ---- interior copies on vector (scalar's ACTIVATE copy is 3x
            # slower). b0 row-split to pipeline with the split DMAs; the
            # first chunk-pair only reads padded rows < 38, i.e. the top
            # copies, so matmuls start before the bottom halves land.
            for b in range(BPC):
                if b == 0:
                    for r0, r1 in ((0, 32), (32, 64)):
                        nc.vector.tensor_copy(
                            xp[b][0:64, PAD + r0 : PAD + r1, PAD : PAD + W],
                            xst[b][0:64, r0:r1, :],
                        )
                        nc.vector.tensor_copy(
                            xp[b][64:128, PAD + r0 : PAD + r1, PAD - 1 : PAD - 1 + W],
                            xst[b][64:128, r0:r1, :],
                        )
                else:
                    nc.vector.tensor_copy(
                        xp[b][0:64, PAD : PAD + H, PAD : PAD + W], xst[b][0:64, :, :]
                    )
                    nc.vector.tensor_copy(
                        xp[b][64:128, PAD : PAD + H, PAD - 1 : PAD - 1 + W],
                        xst[b][64:128, :, :],
                    )
                # xpr: half-A = padded img cols 12.. (img cols 6..63);
                # half-B = same, shifted up one row.
                nc.vector.tensor_copy(
                    xpr[b][0:64, PAD : PAD + H, 0:58], xst[b][0:64, :, 6:64]
                )
                nc.vector.tensor_copy(
                    xpr[b][64:128, PAD - 1 : PAD - 1 + H, 0:58],
                    xst[b][64:128, :, 6:64],
                )

            # ---- main matmul stream ----
            for b in range(BPC):
                for cp in range(NCHUNK // 2):
                    ps = pspool.tile([128, CHUNK], mybir.dt.float32)
                    for t in range(NP):
                        s = _MOD_SLOT.get(t)
                        w_ap = (
                            wm[b][:, s * 64 : (s + 1) * 64]
                            if s is not None
                            else wsh_sb[:, t * 64 : (t + 1) * 64]
                        )
                        if t < 78:
                            row, col, src = t // 6, 2 * (t % 6), xp[b]
                        elif t < 84:
                            row, col, src = 2 * (t - 78), 0, xpr[b]
                        else:
                            row, col, src = 12, 0, xpr[b]
                        for half in (0, 1):
                            r0 = row + 8 * (2 * cp + half)
                            nc.tensor.matmul(
                                ps[64 * half : 64 * (half + 1), :],
                                lhsT=w_ap,
                                rhs=src[:, r0 : r0 + 8, col : col + 64],
                                start=(t == 0),
                                stop=(t == NP - 1),
                                tile_position=(0, 64 * half),
                                skip_group_check=True,
                            )
                    ot = opool.tile([128, CHUNK], f16)
                    nc.vector.tensor_copy(ot[:], ps[:])
                    nc.sync.dma_start(
                        out=y[b, (2 * cp) * 64 : (2 * cp + 2) * 64, :], in_=ot[:]
                    )
    nc.compile()
    return nc


def _get_nc():
    global _CACHED_NC
    if _CACHED_NC is None:
        _CACHED_NC = _build_nc()
    return _CACHED_NC


def _host_dyn(x, w1, b1, w2, b2):
    """dwc_proj MLP on host, float64: dyn [B, 64, 9]."""
    pooled = x[:, :PDIM].mean(axis=(2, 3), dtype=np.float64)      # [B, 64]
    z = pooled @ w1.T.astype(np.float64) + b1.astype(np.float64)  # [B, 32]
    h = 0.5 * z * (1.0 + _ERF(z / math.sqrt(2.0)))                # exact gelu
    dyn = h @ w2.T.astype(np.float64) + b2.astype(np.float64)     # [B, 576]
    return dyn.reshape(B, PDIM, SK * SK)


def _host_weights(lk_filter, dyn):
    """Build shared tap-pair weight tiles + per-batch modified central tiles.

    Weight tile t [128, 64]: rows 0-63 = lk[o, c, iA, jA].T (tap A), rows
    64-127 = tap B, zeros for the lone (12,12) half. lhsT layout [K=c, M=o].
    """
    lkT = lk_filter.transpose(1, 0, 2, 3).astype(np.float32)  # [c, o, i, j]
    Wt = np.zeros((NP, 128, 64), np.float32)
    for i in range(LK):
        for p in range(6):
            t = i * 6 + p
            Wt[t, 0:64, :] = lkT[:, :, i, 2 * p]
            Wt[t, 64:128, :] = lkT[:, :, i, 2 * p + 1]
    for k in range(6):
        Wt[78 + k, 0:64, :] = lkT[:, :, 2 * k, 12]
        Wt[78 + k, 64:128, :] = lkT[:, :, 2 * k + 1, 12]
    Wt[84, 0:64, :] = lkT[:, :, 12, 12]

    ar = np.arange(64)
    Wmod = np.zeros((B, 6, 128, 64), np.float32)
    for ii, i in enumerate((5, 6, 7)):
        t2, t3 = i * 6 + 2, i * 6 + 3
        u = i - 5
        for b in range(B):
            m2 = Wt[t2].copy()
            m3 = Wt[t3].copy()
            m2[64 + ar, ar] += dyn[b, :, u * 3 + 0].astype(np.float32)  # tap (i,5)
            m3[ar, ar] += dyn[b, :, u * 3 + 1].astype(np.float32)       # tap (i,6)
            m3[64 + ar, ar] += dyn[b, :, u * 3 + 2].astype(np.float32)  # tap (i,7)
            Wmod[b, ii] = m2
            Wmod[b, 3 + ii] = m3

    wsh_np = np.ascontiguousarray(
        Wt.transpose(1, 0, 2).reshape(128, NP * 64)
    ).astype(np.float16)
    wmod_np = np.ascontiguousarray(
        Wmod.transpose(0, 2, 1, 3).reshape(B, 128, 6 * 64)
    ).astype(np.float16)
    return wsh_np, wmod_np


def kernel(x, lk_filter, w1, b1, w2, b2):
    from concourse.bass_utils import run_bass_kernel_spmd

    x = np.asarray(x, dtype=np.float32)
    dyn = _host_dyn(x, np.asarray(w1), np.asarray(b1), np.asarray(w2), np.asarray(b2))
    wsh_np, wmod_np = _host_weights(np.asarray(lk_filter, dtype=np.float32), dyn)

    x1_f16 = x[:, :PDIM].astype(np.float16)  # [16, 64, 64, 64]

    nc = _get_nc()
    in_maps = []
    for k in range(NCORES):
        b0 = k * BPC
        in_maps.append(
            {
                "xs": np.ascontiguousarray(x1_f16[b0 : b0 + BPC]),
                "wsh": wsh_np,
                "wmod": np.ascontiguousarray(wmod_np[b0 : b0 + BPC]),
            }
        )
    res = run_bass_kernel_spmd(nc, in_maps, core_ids=list(range(NCORES)))

    out = np.empty((B, C, H, W), np.float32)
    for k in range(NCORES):
        b0 = k * BPC
        yk = res.results[k]["y"].astype(np.float32)          # [BPC, 512, 512]
        yk = yk.reshape(BPC, NCHUNK, 64, CHUNK).transpose(0, 2, 1, 3)
        out[b0 : b0 + BPC, :PDIM] = yk.reshape(BPC, PDIM, H, W)
    out[:, PDIM:] = x[:, PDIM:]
    return out


# revision 10
# speedup vs baseline: 1.0065x; 1.0065x over previous
"""Trainium2 Bass kernel for nn_ConvolutionalAttention_3015067042131.

Math (reference.py):
  x [16,128,64,64] f32; x1 = x[:, :64], x2 = x[:, 64:]
  pooled = mean(x1, HW); h = gelu(pooled @ w1.T + b1); dyn = (h @ w2.T + b2) -> [B,64,9]
  x1_dyn = per-(batch,channel) 3x3 depthwise conv of x1 with dyn
  x1_lk  = conv2d(x1, lk_filter[64,64,13,13], SAME)
  out = concat([x1_lk + x1_dyn, x2], ch)

Strategy:
  * The tiny MLP (dyn) is computed on host in float64 (0.0007% of FLOPs).
  * The dynamic depthwise 3x3 is folded into the 13x13 conv weights as
    per-batch diagonal additions on the central 3x3 taps (3x3 tap (u,v)
    == 13x13 tap (u+5, v+5)); the device runs ONE dense 13x13 conv.
  * Conv as shift-and-matmul with K-dim tap pairing: SBUF partitions
    0-63 hold the zero-padded image, 64-127 hold it shifted one column
    left, so taps (i,2p) and (i,2p+1) fuse into one K=128 matmul
    (78 tiles).  The 13 leftover j=12 taps are paired ROW-wise via a
    second small buffer xpr whose partitions 64-127 hold the padded
    image shifted UP one row: taps (2k,12)+(2k+1,12) fuse (6 tiles),
    (12,12) rides alone (1 tile).  85 tiles total vs the optimal
    ceil(169/2)=85 -> 99.4% K-packing efficiency.
  * Output pixels processed in 8 chunks of 512 (8 rows). Chunk pairs run
    CONCURRENTLY in the two PE column halves via tile_position (0,0) /
    (0,64) writing PSUM partitions 0-63 / 64-127 (measured 2x).
  * fp16 operands (measured end-to-end rel err ~3e-4; f32r is broken in
    this stack and fp32 runs at 1/4 rate). Output stored as f16
    (adds ~<5e-4 elementwise rounding, halves output DMA bytes).
  * Startup critical path (was 24.7us to first real matmul): image DMAs
    are issued BEFORE the weights and split row-wise; weight DMAs are
    issued on the scalar engine's hardware DGE queue so they don't
    serialize behind the image issues on sync; the shared-weight DMA is
    split in 3 so early tap tiles land first; padded-layout interior
    copies run on vector (the scalar-engine copy was 3x slower),
    border memsets on gpsimd/vector.
  * Sharding: data-parallel over batch, 2 batches per core on 8 cores.
    x2 passthrough is host-side (no device work).
"""
import math

import numpy as np

B, C, H, W = 16, 128, 64, 64
PDIM, SK, LK = 64, 3, 13
PAD = LK // 2  # 6
HP, WP = H + 2 * PAD, W + 2 * PAD  # 76, 76
NCORES = 8
BPC = B // NCORES  # batches per core
NP = 85            # 78 col-pair tiles + 6 row-pair j=12 tiles + 1 single
NCHUNK = 8         # 512-pixel chunks per image
CHUNK = H * W // NCHUNK  # 512

# pair tile t = i*6 + p (t<78): taps (i,2p) half-A / (i,2p+1) half-B.
# c12 tile t = 78+k (k<6): taps (2k,12) half-A / (2k+1,12) half-B (xpr).
# t = 84: tap (12,12) half-A only (half-B weight zero).

# central 3x3 taps (i,j in 5..7): j=5 -> half-B of tile i*6+2,
# j=6/7 -> halves A/B of tile i*6+3; those 6 tiles are per-batch.
_MOD_TILES = [5 * 6 + 2, 6 * 6 + 2, 7 * 6 + 2, 5 * 6 + 3, 6 * 6 + 3, 7 * 6 + 3]
_MOD_SLOT = {t: s for s, t in enumerate(_MOD_TILES)}

# wsh DMA split points (tiles): early tap tiles must land first
_WCHUNKS = [(0, 14), (14, 47), (47, NP)]

_ERF = np.vectorize(math.erf, otypes=[np.float64])

_CACHED_NC = None


def _build_nc():
    import concourse.mybir as mybir
    import concourse.tile as tile
    from concourse import bacc

    f32 = mybir.dt.float32
    f16 = mybir.dt.float16

    nc = bacc.Bacc(None, target_bir_lowering=False)
    xs = nc.dram_tensor("xs", [BPC, PDIM, H, W], f16, kind="ExternalInput")
    wsh = nc.dram_tensor("wsh", [128, NP * 64], f16, kind="ExternalInput")
    wmod = nc.dram_tensor("wmod", [BPC, 128, 6 * 64], f16, kind="ExternalInput")
    # chunk-major output: row ci*64+ch holds output rows 8ci..8ci+7 of ch
    y = nc.dram_tensor("y", [BPC, NCHUNK * 64, CHUNK], f16, kind="ExternalOutput")

    with tile.TileContext(nc) as tc:
        with (
            tc.tile_pool(name="wpool", bufs=1) as wpool,
            tc.tile_pool(name="wmpool", bufs=2) as wmpool,
            tc.tile_pool(name="xstpool", bufs=2) as xstpool,
            tc.tile_pool(name="xppool", bufs=2) as xppool,
            tc.tile_pool(name="xprpool", bufs=2) as xprpool,
            tc.tile_pool(name="opool", bufs=3) as opool,
            tc.tile_pool(name="pspool", bufs=4, space="PSUM") as pspool,
        ):
            # ---- warmup scratch (gpsimd memset so vector stays free) ----
            scratch = wpool.tile([128, CHUNK], f16)
            nc.gpsimd.memset(scratch[:], 0.0)

            wsh_sb = wpool.tile([128, NP * 64], f16)
            wm = [wmpool.tile([128, 6 * 64], f16, name=f"wm{b}") for b in range(BPC)]
            xst = [xstpool.tile([128, H, W], f16, name=f"xst{b}") for b in range(BPC)]
            xp = [xppool.tile([128, HP, WP], f16, name=f"xp{b}") for b in range(BPC)]
            xpr = [
                xprpool.tile([128, HP, 64], f16, name=f"xpr{b}") for b in range(BPC)
            ]

            # ---- DMA issue. The DMA engines run at ~1/4 rate until the
            # power manager ramps (~14us), so the startup critical path is
            # the BYTES needed before the first matmul, not the ordering.
            # b0's image is quarter-split row-wise so the first matmuls
            # (which read only the top rows) start after ~256KB; weights
            # issue in parallel on the scalar hardware-DGE queue.
            for q in range(4):
                r0, r1 = 16 * q, 16 * (q + 1)
                nc.sync.dma_start(out=xst[0][0:64, r0:r1, :], in_=xs[0, :, r0:r1, :])
                nc.sync.dma_start(
                    out=xst[0][64:128, r0:r1, :], in_=xs[0, :, r0:r1, :]
                )
            nc.scalar.dma_start(
                out=wsh_sb[:, _WCHUNKS[0][0] * 64 : _WCHUNKS[0][1] * 64],
                in_=wsh[:, _WCHUNKS[0][0] * 64 : _WCHUNKS[0][1] * 64],
            )
            nc.scalar.dma_start(out=wm[0][:], in_=wmod[0, :, :])
            for c0, c1 in _WCHUNKS[1:]:
                nc.scalar.dma_start(
                    out=wsh_sb[:, c0 * 64 : c1 * 64], in_=wsh[:, c0 * 64 : c1 * 64]
                )
            nc.scalar.dma_start(out=xst[1][0:64, :, :], in_=xs[1, :, :, :])
            nc.scalar.dma_start(out=xst[1][64:128, :, :], in_=xs[1, :, :, :])
            nc.scalar.dma_start(out=wm[1][:], in_=wmod[1, :, :])

            # ---- PE warmup: junk matmuls so the HAM clock ramps while the
            # input DMAs are in flight.
            ps_warm = pspool.tile([128, CHUNK], mybir.dt.float32, name="ps_warm", bufs=1)
            for wi in range(4):
                nc.tensor.matmul(
                    ps_warm[0:64, :],
                    lhsT=scratch[:, 0:64],
                    rhs=scratch[:, :],
                    start=(wi == 0),
                    stop=(wi == 3),
                    skip_group_check=True,
                )

            # ---- xp borders; for b0 split vector/gpsimd so the vector is
            # free the moment the image DMA lands.
            def xp_borders(eng, t, eng2=None):
                e2 = eng2 or eng
                eng.memset(t[:, 0:PAD, :], 0.0)
                e2.memset(t[:, PAD + H :, :], 0.0)
                eng.memset(t[0:64, PAD : PAD + H, 0:PAD], 0.0)
                e2.memset(t[0:64, PAD : PAD + H, PAD + W :], 0.0)
                eng.memset(t[64:128, PAD : PAD + H, 0 : PAD - 1], 0.0)
                e2.memset(t[64:128, PAD : PAD + H, PAD - 1 + W :], 0.0)

            def xpr_borders(eng, t):
                # half-A: padded rows at col offset 12; half-B same shifted
                # up one row.  cols 58:64 map past the padded width -> zero.
                eng.memset(t[0:64, 0:PAD, :], 0.0)
                eng.memset(t[0:64, PAD + H :, :], 0.0)
                eng.memset(t[0:64, PAD : PAD + H, 58:64], 0.0)
                eng.memset(t[64:128, 0 : PAD - 1, :], 0.0)
                eng.memset(t[64:128, PAD - 1 + H :, :], 0.0)
                eng.memset(t[64:128, PAD - 1 : PAD - 1 + H, 58:64], 0.0)

            xp_borders(nc.vector, xp[0], eng2=nc.gpsimd)
            # remaining borders are needed much later -> gpsimd
            xpr_borders(nc.gpsimd, xpr[0])
            xp_borders(nc.gpsimd, xp[1])
            xpr_borders(nc.gpsimd, xpr[1])

            # ---- interior copies on vector (scalar's ACTIVATE copy is 3x
            # slower). b0 row-split to pipeline with the split DMAs; the
            # first chunk-pair only reads padded rows < 38, i.e. the top
            # copies, so matmuls start before the bottom halves land.
            for b in range(BPC):
                if b == 0:
                    for q in range(4):
                        r0, r1 = 16 * q, 16 * (q + 1)
                        nc.vector.tensor_copy(
                            xp[b][0:64, PAD + r0 : PAD + r1, PAD : PAD + W],
                            xst[b][0:64, r0:r1, :],
                        )
                        nc.vector.tensor_copy(
                            xp[b][64:128, PAD + r0 : PAD + r1, PAD - 1 : PAD - 1 + W],
                            xst[b][64:128, r0:r1, :],
                        )
                else:
                    nc.vector.tensor_copy(
                        xp[b][0:64, PAD : PAD + H, PAD : PAD + W], xst[b][0:64, :, :]
                    )
                    nc.vector.tensor_copy(
                        xp[b][64:128, PAD : PAD + H, PAD - 1 : PAD - 1 + W],
                        xst[b][64:128, :, :],
                    )
                # xpr: half-A = padded img cols 12.. (img cols 6..63);
                # half-B = same, shifted up one row.
                nc.vector.tensor_copy(
                    xpr[b][0:64, PAD : PAD + H, 0:58], xst[b][0:64, :, 6:64]
                )
                nc.vector.tensor_copy(
                    xpr[b][64:128, PAD - 1 : PAD - 1 + H, 0:58],
                    xst[b][64:128, :, 6:64],
                )

            # ---- main matmul stream ----
            for b in range(BPC):
                for cp in range(NCHUNK // 2):
                    ps = pspool.tile([128, CHUNK], mybir.dt.float32)
                    for t in range(NP):
                        s = _MOD_SLOT.get(t)
                        w_ap = (
                            wm[b][:, s * 64 : (s + 1) * 64]
                            if s is not None
                            else wsh_sb[:, t * 64 : (t + 1) * 64]
                        )
                        if t < 78:
                            row, col, src = t // 6, 2 * (t % 6), xp[b]
                        elif t < 84:
                            row, col, src = 2 * (t - 78), 0, xpr[b]
                        else:
                            row, col, src = 12, 0, xpr[b]
                        for half in (0, 1):
                            r0 = row + 8 * (2 * cp + half)
                            nc.tensor.matmul(
                                ps[64 * half : 64 * (half + 1), :],
                                lhsT=w_ap,
                                rhs=src[:, r0 : r0 + 8, col : col + 64],
                                start=(t == 0),
                                stop=(t == NP - 1),
                                tile_position=(0, 64 * half),
                                skip_group_check=True,
                            )
                    ot = opool.tile([128, CHUNK], f16)
                    nc.vector.tensor_copy(ot[:], ps[:])
                    nc.sync.dma_start(
                        out=y[b, (2 * cp) * 64 : (2 * cp + 2) * 64, :], in_=ot[:]
                    )
    nc.compile()
    return nc


def _get_nc():
    global _CACHED_NC
    if _CACHED_NC is None:
        _CACHED_NC = _build_nc()
    return _CACHED_NC


def _host_dyn(x, w1, b1, w2, b2):
    """dwc_proj MLP on host, float64: dyn [B, 64, 9]."""
    pooled = x[:, :PDIM].mean(axis=(2, 3), dtype=np.float64)      # [B, 64]
    z = pooled @ w1.T.astype(np.float64) + b1.astype(np.float64)  # [B, 32]
    h = 0.5 * z * (1.0 + _ERF(z / math.sqrt(2.0)))                # exact gelu
    dyn = h @ w2.T.astype(np.float64) + b2.astype(np.float64)     # [B, 576]
    return dyn.reshape(B, PDIM, SK * SK)


def _host_weights(lk_filter, dyn):
    """Build shared tap-pair weight tiles + per-batch modified central tiles.

    Weight tile t [128, 64]: rows 0-63 = lk[o, c, iA, jA].T (tap A), rows
    64-127 = tap B, zeros for the lone (12,12) half. lhsT layout [K=c, M=o].
    """
    lkT = lk_filter.transpose(1, 0, 2, 3).astype(np.float32)  # [c, o, i, j]
    Wt = np.zeros((NP, 128, 64), np.float32)
    for i in range(LK):
        for p in range(6):
            t = i * 6 + p
            Wt[t, 0:64, :] = lkT[:, :, i, 2 * p]
            Wt[t, 64:128, :] = lkT[:, :, i, 2 * p + 1]
    for k in range(6):
        Wt[78 + k, 0:64, :] = lkT[:, :, 2 * k, 12]
        Wt[78 + k, 64:128, :] = lkT[:, :, 2 * k + 1, 12]
    Wt[84, 0:64, :] = lkT[:, :, 12, 12]

    ar = np.arange(64)
    Wmod = np.zeros((B, 6, 128, 64), np.float32)
    for ii, i in enumerate((5, 6, 7)):
        t2, t3 = i * 6 + 2, i * 6 + 3
        u = i - 5
        for b in range(B):
            m2 = Wt[t2].copy()
            m3 = Wt[t3].copy()
            m2[64 + ar, ar] += dyn[b, :, u * 3 + 0].astype(np.float32)  # tap (i,5)
            m3[ar, ar] += dyn[b, :, u * 3 + 1].astype(np.float32)       # tap (i,6)
            m3[64 + ar, ar] += dyn[b, :, u * 3 + 2].astype(np.float32)  # tap (i,7)
            Wmod[b, ii] = m2
            Wmod[b, 3 + ii] = m3

    wsh_np = np.ascontiguousarray(
        Wt.transpose(1, 0, 2).reshape(128, NP * 64)
    ).astype(np.float16)
    wmod_np = np.ascontiguousarray(
        Wmod.transpose(0, 2, 1, 3).reshape(B, 128, 6 * 64)
    ).astype(np.float16)
    return wsh_np, wmod_np


def kernel(x, lk_filter, w1, b1, w2, b2):
    from concourse.bass_utils import run_bass_kernel_spmd

    x = np.asarray(x, dtype=np.float32)
    dyn = _host_dyn(x, np.asarray(w1), np.asarray(b1), np.asarray(w2), np.asarray(b2))
    wsh_np, wmod_np = _host_weights(np.asarray(lk_filter, dtype=np.float32), dyn)

    x1_f16 = x[:, :PDIM].astype(np.float16)  # [16, 64, 64, 64]

    nc = _get_nc()
    in_maps = []
    for k in range(NCORES):
        b0 = k * BPC
        in_maps.append(
            {
                "xs": np.ascontiguousarray(x1_f16[b0 : b0 + BPC]),
                "wsh": wsh_np,
                "wmod": np.ascontiguousarray(wmod_np[b0 : b0 + BPC]),
            }
        )
    res = run_bass_kernel_spmd(nc, in_maps, core_ids=list(range(NCORES)))

    out = np.empty((B, C, H, W), np.float32)
    for k in range(NCORES):
        b0 = k * BPC
        yk = res.results[k]["y"].astype(np.float32)          # [BPC, 512, 512]
        yk = yk.reshape(BPC, NCHUNK, 64, CHUNK).transpose(0, 2, 1, 3)
        out[b0 : b0 + BPC, :PDIM] = yk.reshape(BPC, PDIM, H, W)
    out[:, PDIM:] = x[:, PDIM:]
    return out


# revision 11
# speedup vs baseline: 1.0098x; 1.0032x over previous
"""Trainium2 Bass kernel for nn_ConvolutionalAttention_3015067042131.

Math (reference.py):
  x [16,128,64,64] f32; x1 = x[:, :64], x2 = x[:, 64:]
  pooled = mean(x1, HW); h = gelu(pooled @ w1.T + b1); dyn = (h @ w2.T + b2) -> [B,64,9]
  x1_dyn = per-(batch,channel) 3x3 depthwise conv of x1 with dyn
  x1_lk  = conv2d(x1, lk_filter[64,64,13,13], SAME)
  out = concat([x1_lk + x1_dyn, x2], ch)

Strategy:
  * The tiny MLP (dyn) is computed on host in float64 (0.0007% of FLOPs).
  * The dynamic depthwise 3x3 is folded into the 13x13 conv weights as
    per-batch diagonal additions on the central 3x3 taps (3x3 tap (u,v)
    == 13x13 tap (u+5, v+5)); the device runs ONE dense 13x13 conv.
  * Conv as shift-and-matmul with K-dim tap pairing: SBUF partitions
    0-63 hold the zero-padded image, 64-127 hold it shifted one column
    left, so taps (i,2p) and (i,2p+1) fuse into one K=128 matmul
    (78 tiles).  The 13 leftover j=12 taps are paired ROW-wise via a
    second buffer xpr whose partitions 64-127 hold the padded image
    shifted UP one row: taps (2k,12)+(2k+1,12) fuse (6 tiles), (12,12)
    rides alone (1 tile).  85 tiles = ceil(169/2) -> optimal K packing.
  * Output pixels processed in 8 chunks of 512 (8 rows). Chunk pairs run
    CONCURRENTLY in the two PE column halves via tile_position (0,0) /
    (0,64) writing PSUM partitions 0-63 / 64-127 (measured 2x).
  * fp16 operands (measured end-to-end rel err ~3e-4; f32r is broken in
    this stack and fp32 runs at 1/4 rate). Output stored as f16.
  * The padded+shifted layouts are built ON HOST and DMA'd directly into
    SBUF as contiguous per-partition runs: no staging copies, no border
    memsets, no DVE work on the critical path, and far fewer
    cross-engine events (the event teardown at kernel end costs ~115ns
    per event on the tensor engine).
  * The DMA engines run at ~1/4 rate until the power manager ramps
    (~14us), so b0's padded image is row-quartered and the first
    quarter (rows 0-21, enough for the first ~40 weight tiles) is
    issued first; weights stream in parallel on the scalar DGE queue.
  * Sharding: data-parallel over batch, 2 batches per core on 8 cores.
    x2 passthrough is host-side (no device work).
"""
import math

import numpy as np

B, C, H, W = 16, 128, 64, 64
PDIM, SK, LK = 64, 3, 13
PAD = LK // 2  # 6
HP, WP = H + 2 * PAD, W + 2 * PAD  # 76, 76
NCORES = 8
BPC = B // NCORES  # batches per core
NP = 85            # 78 col-pair tiles + 6 row-pair j=12 tiles + 1 single
NCHUNK = 8         # 512-pixel chunks per image
CHUNK = H * W // NCHUNK  # 512

# pair tile t = i*6 + p (t<78): taps (i,2p) half-A / (i,2p+1) half-B.
# c12 tile t = 78+k (k<6): taps (2k,12) half-A / (2k+1,12) half-B (xpr).
# t = 84: tap (12,12) half-A only (half-B weight zero).

# central 3x3 taps (i,j in 5..7): j=5 -> half-B of tile i*6+2,
# j=6/7 -> halves A/B of tile i*6+3; those 6 tiles are per-batch.
_MOD_TILES = [5 * 6 + 2, 6 * 6 + 2, 7 * 6 + 2, 5 * 6 + 3, 6 * 6 + 3, 7 * 6 + 3]
_MOD_SLOT = {t: s for s, t in enumerate(_MOD_TILES)}

# wsh DMA split points (tiles): early tap tiles must land first
_WCHUNKS = [(0, 14), (14, 47), (47, NP)]
# b0 padded-image row quarters (in padded row coords)
_RQUARTERS = [(0, 22), (22, 40), (40, 58), (58, HP)]

_ERF = np.vectorize(math.erf, otypes=[np.float64])

_CACHED_NC = None


def _build_nc():
    import concourse.mybir as mybir
    import concourse.tile as tile
    from concourse import bacc

    f16 = mybir.dt.float16

    nc = bacc.Bacc(None, target_bir_lowering=False)
    xpd = nc.dram_tensor("xpd", [BPC, 128, HP, WP], f16, kind="ExternalInput")
    xprd = nc.dram_tensor("xprd", [BPC, 128, HP, 64], f16, kind="ExternalInput")
    wsh = nc.dram_tensor("wsh", [128, NP * 64], f16, kind="ExternalInput")
    wmod = nc.dram_tensor("wmod", [BPC, 128, 6 * 64], f16, kind="ExternalInput")
    # chunk-major output: row ci*64+ch holds output rows 8ci..8ci+7 of ch
    y = nc.dram_tensor("y", [BPC, NCHUNK * 64, CHUNK], f16, kind="ExternalOutput")

    with tile.TileContext(nc) as tc:
        with (
            tc.tile_pool(name="wpool", bufs=1) as wpool,
            tc.tile_pool(name="wmpool", bufs=2) as wmpool,
            tc.tile_pool(name="xppool", bufs=2) as xppool,
            tc.tile_pool(name="xprpool", bufs=2) as xprpool,
            tc.tile_pool(name="opool", bufs=3) as opool,
            tc.tile_pool(name="pspool", bufs=4, space="PSUM") as pspool,
        ):
            # ---- warmup scratch (gpsimd memset; gpsimd is otherwise idle)
            scratch = wpool.tile([128, CHUNK], f16)
            nc.gpsimd.memset(scratch[:], 0.0)

            wsh_sb = wpool.tile([128, NP * 64], f16)
            wm = [wmpool.tile([128, 6 * 64], f16, name=f"wm{b}") for b in range(BPC)]
            xp = [xppool.tile([128, HP, WP], f16, name=f"xp{b}") for b in range(BPC)]
            xpr = [
                xprpool.tile([128, HP, 64], f16, name=f"xpr{b}") for b in range(BPC)
            ]

            # ---- DMA issue: b0 image quarters on sync, weights in
            # parallel on the scalar hardware-DGE queue.
            for r0, r1 in _RQUARTERS:
                nc.sync.dma_start(out=xp[0][:, r0:r1, :], in_=xpd[0, :, r0:r1, :])
            nc.scalar.dma_start(
                out=wsh_sb[:, _WCHUNKS[0][0] * 64 : _WCHUNKS[0][1] * 64],
                in_=wsh[:, _WCHUNKS[0][0] * 64 : _WCHUNKS[0][1] * 64],
            )
            nc.scalar.dma_start(out=wm[0][:], in_=wmod[0, :, :])
            nc.sync.dma_start(out=xpr[0][:], in_=xprd[0, :, :, :])
            for c0, c1 in _WCHUNKS[1:]:
                nc.scalar.dma_start(
                    out=wsh_sb[:, c0 * 64 : c1 * 64], in_=wsh[:, c0 * 64 : c1 * 64]
                )
            nc.sync.dma_start(out=xp[1][:], in_=xpd[1, :, :, :])
            nc.sync.dma_start(out=xpr[1][:], in_=xprd[1, :, :, :])
            nc.scalar.dma_start(out=wm[1][:], in_=wmod[1, :, :])

            # ---- PE warmup: junk matmuls so the clock ramps while the
            # input DMAs are in flight.
            ps_warm = pspool.tile([128, CHUNK], mybir.dt.float32, name="ps_warm", bufs=1)
            for wi in range(4):
                nc.tensor.matmul(
                    ps_warm[0:64, :],
                    lhsT=scratch[:, 0:64],
                    rhs=scratch[:, :],
                    start=(wi == 0),
                    stop=(wi == 3),
                    skip_group_check=True,
                )

            # ---- main matmul stream ----
            for b in range(BPC):
                for cp in range(NCHUNK // 2):
                    ps = pspool.tile([128, CHUNK], mybir.dt.float32)
                    for t in range(NP):
                        s = _MOD_SLOT.get(t)
                        w_ap = (
                            wm[b][:, s * 64 : (s + 1) * 64]
                            if s is not None
                            else wsh_sb[:, t * 64 : (t + 1) * 64]
                        )
                        if t < 78:
                            row, col, src = t // 6, 2 * (t % 6), xp[b]
                        elif t < 84:
                            row, col, src = 2 * (t - 78), 0, xpr[b]
                        else:
                            row, col, src = 12, 0, xpr[b]
                        for half in (0, 1):
                            r0 = row + 8 * (2 * cp + half)
                            nc.tensor.matmul(
                                ps[64 * half : 64 * (half + 1), :],
                                lhsT=w_ap,
                                rhs=src[:, r0 : r0 + 8, col : col + 64],
                                start=(t == 0),
                                stop=(t == NP - 1),
                                tile_position=(0, 64 * half),
                                skip_group_check=True,
                            )
                    ot = opool.tile([128, CHUNK], f16)
                    nc.vector.tensor_copy(ot[:], ps[:])
                    nc.sync.dma_start(
                        out=y[b, (2 * cp) * 64 : (2 * cp + 2) * 64, :], in_=ot[:]
                    )
    nc.compile()
    return nc


def _get_nc():
    global _CACHED_NC
    if _CACHED_NC is None:
        _CACHED_NC = _build_nc()
    return _CACHED_NC


def _host_dyn(x, w1, b1, w2, b2):
    """dwc_proj MLP on host, float64: dyn [B, 64, 9]."""
    pooled = x[:, :PDIM].mean(axis=(2, 3), dtype=np.float64)      # [B, 64]
    z = pooled @ w1.T.astype(np.float64) + b1.astype(np.float64)  # [B, 32]
    h = 0.5 * z * (1.0 + _ERF(z / math.sqrt(2.0)))                # exact gelu
    dyn = h @ w2.T.astype(np.float64) + b2.astype(np.float64)     # [B, 576]
    return dyn.reshape(B, PDIM, SK * SK)


def _host_weights(lk_filter, dyn):
    """Build shared tap-pair weight tiles + per-batch modified central tiles.

    Weight tile t [128, 64]: rows 0-63 = lk[o, c, iA, jA].T (tap A), rows
    64-127 = tap B, zeros for the lone (12,12) half. lhsT layout [K=c, M=o].
    """
    lkT = lk_filter.transpose(1, 0, 2, 3).astype(np.float32)  # [c, o, i, j]
    Wt = np.zeros((NP, 128, 64), np.float32)
    for i in range(LK):
        for p in range(6):
            t = i * 6 + p
            Wt[t, 0:64, :] = lkT[:, :, i, 2 * p]
            Wt[t, 64:128, :] = lkT[:, :, i, 2 * p + 1]
    for k in range(6):
        Wt[78 + k, 0:64, :] = lkT[:, :, 2 * k, 12]
        Wt[78 + k, 64:128, :] = lkT[:, :, 2 * k + 1, 12]
    Wt[84, 0:64, :] = lkT[:, :, 12, 12]

    ar = np.arange(64)
    Wmod = np.zeros((B, 6, 128, 64), np.float32)
    for ii, i in enumerate((5, 6, 7)):
        t2, t3 = i * 6 + 2, i * 6 + 3
        u = i - 5
        for b in range(B):
            m2 = Wt[t2].copy()
            m3 = Wt[t3].copy()
            m2[64 + ar, ar] += dyn[b, :, u * 3 + 0].astype(np.float32)  # tap (i,5)
            m3[ar, ar] += dyn[b, :, u * 3 + 1].astype(np.float32)       # tap (i,6)
            m3[64 + ar, ar] += dyn[b, :, u * 3 + 2].astype(np.float32)  # tap (i,7)
            Wmod[b, ii] = m2
            Wmod[b, 3 + ii] = m3

    wsh_np = np.ascontiguousarray(
        Wt.transpose(1, 0, 2).reshape(128, NP * 64)
    ).astype(np.float16)
    wmod_np = np.ascontiguousarray(
        Wmod.transpose(0, 2, 1, 3).reshape(B, 128, 6 * 64)
    ).astype(np.float16)
    return wsh_np, wmod_np


def _host_images(x1_f16):
    """Padded + shifted device layouts, built on host.

    xpd [B,128,HP,WP]: partitions 0-63 zero-padded image, 64-127 the same
    shifted one column LEFT.  xprd [B,128,HP,64]: partitions 0-63 padded
    cols 12.., 64-127 the same shifted one row UP.
    """
    P = np.zeros((B, 64, HP, WP), np.float16)
    P[:, :, PAD : PAD + H, PAD : PAD + W] = x1_f16
    xpd = np.empty((B, 128, HP, WP), np.float16)
    xpd[:, 0:64] = P
    xpd[:, 64:128, :, : WP - 1] = P[:, :, :, 1:]
    xpd[:, 64:128, :, WP - 1] = 0.0
    xprd = np.empty((B, 128, HP, 64), np.float16)
    xprd[:, 0:64] = P[:, :, :, 12 : 12 + 64]
    xprd[:, 64:128, : HP - 1, :] = P[:, :, 1:, 12 : 12 + 64]
    xprd[:, 64:128, HP - 1, :] = 0.0
    return xpd, xprd


def _host_inmaps(x, lk_filter, w1, b1, w2, b2):
    """Full input prep -> per-core in_maps for run_bass_kernel_spmd."""
    x = np.asarray(x, dtype=np.float32)
    dyn = _host_dyn(x, np.asarray(w1), np.asarray(b1), np.asarray(w2), np.asarray(b2))
    wsh_np, wmod_np = _host_weights(np.asarray(lk_filter, dtype=np.float32), dyn)
    xpd, xprd = _host_images(x[:, :PDIM].astype(np.float16))
    in_maps = []
    for k in range(NCORES):
        b0 = k * BPC
        in_maps.append(
            {
                "xpd": np.ascontiguousarray(xpd[b0 : b0 + BPC]),
                "xprd": np.ascontiguousarray(xprd[b0 : b0 + BPC]),
                "wsh": wsh_np,
                "wmod": np.ascontiguousarray(wmod_np[b0 : b0 + BPC]),
            }
        )
    return in_maps


def kernel(x, lk_filter, w1, b1, w2, b2):
    from concourse.bass_utils import run_bass_kernel_spmd

    x = np.asarray(x, dtype=np.float32)
    in_maps = _host_inmaps(x, lk_filter, w1, b1, w2, b2)
    nc = _get_nc()
    res = run_bass_kernel_spmd(nc, in_maps, core_ids=list(range(NCORES)))

    out = np.empty((B, C, H, W), np.float32)
    for k in range(NCORES):
        b0 = k * BPC
        yk = res.results[k]["y"].astype(np.float32)          # [BPC, 512, 512]
        yk = yk.reshape(BPC, NCHUNK, 64, CHUNK).transpose(0, 2, 1, 3)
        out[b0 : b0 + BPC, :PDIM] = yk.reshape(BPC, PDIM, H, W)
    out[:, PDIM:] = x[:, PDIM:]
    return out
